# revision 17
# baseline (speedup 1.0000x reference)
"""GNN bi-interaction aggregator (gnn_message_passing) on 8 trn2 NeuronCores.

reference:
    msgs = edge_val[:, None] * embeddings[edge_col]          # [E, D]
    side = segment_sum(msgs, edge_row, N)                    # [N, D]
    out  = lrelu((emb + side) @ W_sum.T + b_sum)
         + lrelu((emb * side) @ W_prod.T + b_prod)

Sharding: row-partition destinations across 8 cores (6250 nodes each); each
core receives the full embedding table (bf16) in its DRAM plus its own edge
shard, so no collectives are needed.

Per-core algorithm:
  - host sorts the core's edges by destination into 128-segment windows and
    into two source classes (src < 32768 / >= 32768, the int16 dma_gather
    limit), padded to 128-edge tiles with a cross-core-uniform tile schedule
    (SPMD: one program for all cores; counts are max over cores).
  - chunked dma_gather pulls source rows (bf16, 256B/row) into SBUF; per
    chunk 4 gather calls round-robin the 4 SWDGE queues with class B
    rotating so per-queue byte load balances; the SWDGE descriptor ring
    (dynamic_dma_scratch_size) is enlarged to reduce mid-call blocking.
  - the scatter matrix S[slot, seg] = val[slot] * (destrel[slot] == seg) is
    host-precomputed (bf16) and streamed per chunk (on-chip builds are
    DVE-bound: ~115 elem/ns for 2-input DVE ops makes them ~3x slower than
    streaming 22MB via DMA).
  - per window, PE accumulates sideT[d, seg] += msgs.T @ S in a PSUM bank
    (start/stop accumulation groups); ACT drains to bf16; DVE forms
    X1 = embT + side, X2 = embT * side per window.
  - software pipeline: chunk k gathers + S streams, chunk k-1 scatter
    matmuls + drains, chunk k-2 downstream (node-major matmuls with
    lhsT = X1/X2, rhs = W.T; Lrelu on ACT; add on DVE; store).
"""
import math
import numpy as np
import ml_dtypes

import concourse.bass as bass
import concourse.bacc as bacc
import concourse.mybir as mybir
import concourse.tile as tile
from concourse.bass_utils import run_bass_kernel_spmd
from concourse.masks import make_identity

N_NODES = 50000
N_EDGES = 600000
D = 128
NCORES = 8
NPC = N_NODES // NCORES          # 6250 destinations per core
WSEG = 128                       # segments per PSUM window
NW = math.ceil(NPC / WSEG)       # 49 windows per core
SPLIT = 32768                    # source class boundary (int16 idx limit)
CHUNK_W = 5                      # max windows per gather chunk
# ragged chunk sizes: small first chunk fills the pipeline fast, small
# last chunks shorten the drain tail
CHUNKS = [1, 3, 4, 5, 5, 5, 5, 5, 5, 5, 4, 2]
assert sum(CHUNKS) == NW
W_LO = np.concatenate([[0], np.cumsum(CHUNKS)])
NCHUNK = len(CHUNKS)
SCRATCH = 49152                  # SWDGE descriptor ring bytes/partition
SLOPE = 0.01
BF16 = ml_dtypes.bfloat16
SIM_RELU = False   # CoreSim lacks Lrelu; set True only in sim tests


def _wrap_idx(idx: np.ndarray) -> np.ndarray:
    """dma_gather idx layout: arr[p, s] = idx[s*16 + p%16], [128, n/16] int16."""
    n = len(idx)
    assert n % 16 == 0
    a16 = idx.astype(np.int16).reshape(-1, 16).T          # [16, n/16]
    return np.ascontiguousarray(np.tile(a16, (8, 1)))     # [128, n/16]


def _assign_dests(edge_row):
    """Degree-sorted serpentine destination->core assignment.

    Equalizes per-(window, core) edge counts so the SPMD max-over-cores tile
    schedule has near-minimal padding. Returns (dest2core, dest2slot,
    core_dests) where core_dests[c] lists the global dest ids owned by core c
    in slot order (used for emb_own layout and output reassembly).
    """
    tot = np.bincount(edge_row, minlength=N_NODES)
    order = np.argsort(-tot, kind="stable")
    j = np.arange(N_NODES)
    rnd = j // NCORES                     # slot index within core
    pos = j % NCORES
    corej = np.where((rnd % 2) == 0, pos, NCORES - 1 - pos)
    dest2core = np.empty(N_NODES, np.int64)
    dest2slot = np.empty(N_NODES, np.int64)
    dest2core[order] = corej
    dest2slot[order] = rnd
    core_dests = np.empty((NCORES, NPC), np.int64)
    core_dests[corej, rnd] = order
    return dest2core, dest2slot, core_dests


def _preprocess(edge_row, edge_col, edge_val):
    """Index-only preprocessing -> per-core arrays + straddle-tile schedule.

    Slots are packed contiguously per class (no per-window rounding); the
    compile-time schedule matmuls window w over the tile span
    [ts(w), te(w)) derived from min/max-over-cores cumulative counts.
    Boundary tiles appear in two windows' spans with complementary
    nonzeros in S; span-extra tiles get all-zero S columns.
    """
    dest2core, dest2slot, core_dests = _assign_dests(edge_row)
    core = dest2core[edge_row]
    er = dest2slot[edge_row]             # dest local to core
    w = er // WSEG                       # window id
    destrel = er - w * WSEG              # seg within window, [0, 128)
    cls = (edge_col >= SPLIT).astype(np.int64)   # 0=A, 1=B

    key = (core * 2 + cls) * NW + w
    counts = np.bincount(key, minlength=NCORES * 2 * NW).reshape(NCORES, 2, NW)
    cums = np.concatenate([np.zeros((NCORES, 2, 1), np.int64),
                           np.cumsum(counts, axis=2)], axis=2)
    sched = {}
    for x in range(2):
        Sx = int(np.ceil(cums[:, x, -1].max() / 128))
        ts = np.floor(cums[:, x, :-1].min(axis=0) / 128).astype(np.int64)
        te = np.ceil(cums[:, x, 1:].max(axis=0) / 128).astype(np.int64)
        te = np.maximum(te, ts + 1)
        pair_base = np.concatenate([[0], np.cumsum(te - ts)])
        sched[x] = dict(Sx=Sx, ts=ts, te=te, pair_base=pair_base,
                        npairs=int(pair_base[-1]))

    per_core = []
    for c in range(NCORES):
        m = core == c
        ccls, cw = cls[m], w[m]
        csrc, cdr, cval = edge_col[m], destrel[m], edge_val[m]
        s_parts = []
        idxs = []
        for x in range(2):
            sc = sched[x]
            mm = ccls == x
            xw, xsrc = cw[mm], csrc[mm]
            xdr, xval = cdr[mm], cval[mm]
            order = np.argsort(xw, kind="stable")
            xw, xsrc, xdr, xval = xw[order], xsrc[order], xdr[order], xval[order]
            n = len(xw)
            idx = np.zeros(sc["Sx"] * 128, np.int64)
            idx[:n] = xsrc - (SPLIT if x else 0)
            idxs.append(idx)
            slot = np.arange(n)
            tile_of = slot // 128
            pair = sc["pair_base"][xw] + (tile_of - sc["ts"][xw])
            S = np.zeros((128, sc["npairs"] * WSEG), np.float32)
            S[slot % 128, pair * WSEG + xdr] = xval
            s_parts.append(S)
        S_cols = np.concatenate(s_parts, axis=1).astype(BF16)
        per_core.append(dict(
            idxA=_wrap_idx(idxs[0]),
            idxB=_wrap_idx(idxs[1]),
            S=np.ascontiguousarray(S_cols),
        ))
    return per_core, sched, core_dests


def _act_kw():
    if SIM_RELU:
        return dict(func=mybir.ActivationFunctionType.Relu)
    return dict(func=mybir.ActivationFunctionType.Lrelu, alpha=SLOPE)


def _build(nc, sched, with_bias):
    scA, scB = sched[0], sched[1]
    SxA, SxB = scA["Sx"], scB["Sx"]
    npA, npB = scA["npairs"], scB["npairs"]
    bf = mybir.dt.bfloat16
    f32 = mybir.dt.float32

    t_table = nc.declare_dram_parameter("table", [N_NODES, D], bf, isOutput=False)
    t_emb = nc.declare_dram_parameter("emb_own", [NPC, D], bf, isOutput=False)
    t_idxA = nc.declare_dram_parameter("idxA", [128, SxA * 8],
                                       mybir.dt.int16, isOutput=False)
    t_idxB = nc.declare_dram_parameter("idxB", [128, SxB * 8],
                                       mybir.dt.int16, isOutput=False)
    t_S = nc.declare_dram_parameter("S", [128, (npA + npB) * WSEG], bf,
                                    isOutput=False)
    t_wsum = nc.declare_dram_parameter("wsumT", [D, D], bf, isOutput=False)
    t_wprod = nc.declare_dram_parameter("wprodT", [D, D], bf, isOutput=False)
    if with_bias:
        t_bias = nc.declare_dram_parameter("biases", [128, 2 * D], f32,
                                           isOutput=False)
    t_out = nc.declare_dram_parameter("out", [NPC, D], f32, isOutput=True)

    NCOL = NW * WSEG  # 6272 columns in T-space buffers

    # disjoint per-chunk gather tile ranges; matmuls may reach back one
    # chunk for straddle tiles
    g = np.zeros((2, NCHUNK + 1), dtype=np.int64)
    for x, sc in ((0, scA), (1, scB)):
        for k in range(NCHUNK):
            g[x, k + 1] = sc["te"][W_LO[k + 1] - 1]
        g[x, NCHUNK] = sc["Sx"]
        for k in range(1, NCHUNK):
            assert sc["ts"][W_LO[k]] >= g[x, k - 1], (x, k)

    with tile.TileContext(nc) as tc:
        with (
            tc.tile_pool(name="cst", bufs=1) as cst,
            tc.tile_pool(name="gbufA", bufs=4) as gbufA,
            tc.tile_pool(name="gbufB", bufs=4) as gbufB,
            tc.tile_pool(name="idxp", bufs=1) as idxp,
            tc.tile_pool(name="sSA", bufs=2) as sSA,
            tc.tile_pool(name="sSB", bufs=2) as sSB,
            tc.tile_pool(name="braw", bufs=4) as brawp,
            tc.tile_pool(name="px1", bufs=2 * CHUNK_W + 1) as px1p,
            tc.tile_pool(name="px2", bufs=2 * CHUNK_W + 1) as px2p,
            tc.tile_pool(name="small", bufs=3) as small,
            tc.tile_pool(name="psw", bufs=3, space="PSUM") as psw,
            tc.tile_pool(name="psd", bufs=3, space="PSUM") as psd,
            tc.tile_pool(name="pst", bufs=2, space="PSUM") as pst,
        ):
            srcA = t_table[:SPLIT, :]
            srcB = t_table[SPLIT:, :]

            # per-chunk state for the software pipeline
            chunk_bufs = {}   # k -> (bufs[2], sbufs[2])
            win_x = {}        # w -> (x1 tile, x2 tile)

            # idx tiles: a tiny chunk-0 pair (first DMAs, on the idle ACT
            # hwdge queue) plus one bulk pair for chunks 1+, so chunk-0
            # gathers never wait on the bulk load and chunk-1+ gathers wait
            # on a single early DMA rather than per-chunk transfers stuck
            # behind S-stream traffic.
            idx0 = []
            idxR = []
            for x, t_idx, Sx in ((0, t_idxA, SxA), (1, t_idxB, SxB)):
                c0 = int(g[x, 1]) * 8
                it0 = idxp.tile([128, c0], mybir.dt.int16, tag=f"ix0_{x}")
                nc.scalar.dma_start(out=it0[:], in_=t_idx[:, :c0])
                idx0.append(it0)
                idxR.append((c0, Sx * 8))

            def emit_idx_rest():
                for x, t_idx in ((0, t_idxA), (1, t_idxB)):
                    c0, hi = idxR[x]
                    it = idxp.tile([128, hi - c0], mybir.dt.int16,
                                   tag=f"ixR_{x}")
                    nc.scalar.dma_start(out=it[:], in_=t_idx[:, c0:])
                    idxR[x] = (c0, it)

            def emit_gather_and_s(k):
                ntA = int(g[0, k + 1] - g[0, k])
                ntB = int(g[1, k + 1] - g[1, k])
                assert ntA >= 3 and ntB >= 1, (k, ntA, ntB)
                bufs = [
                    gbufA.tile([128, ntA, D], bf, tag="g0", name="gbuf0"),
                    gbufB.tile([128, ntB, D], bf, tag="g1", name="gbuf1"),
                ]
                # 8 gather pieces: each queue gets 1/4 of class A plus 1/4
                # of class B, so per-queue idx counts (the async Q7 prep
                # load) balance exactly; A pieces issue first.
                for x, nt in ((0, ntA), (1, ntB)):
                    bounds = [round(i * nt / 4) for i in range(5)]
                    for qi in range(4):
                        t0, t1 = bounds[qi], bounds[qi + 1]
                        if t1 == t0:
                            continue
                        if k == 0:
                            idxt = idx0[x]
                            co = t0 * 8
                        else:
                            c0, idxt = idxR[x]
                            co = (int(g[x, k]) + t0) * 8 - c0
                        srct = (srcA, srcB)[x]
                        n_idx = (t1 - t0) * 128
                        nc.gpsimd.dma_gather(
                            bufs[x][:, t0:t1, :], srct,
                            idxt[:, co : co + (t1 - t0) * 8],
                            n_idx, n_idx, D, single_packet=False,
                            queue_num=qi,
                        )
                # stream host-precomputed S pair-columns for this chunk
                sbufs = []
                for x, sc, pool in ((0, scA, sSA), (1, scB, sSB)):
                    p_lo = int(sc["pair_base"][W_LO[k]])
                    p_hi = int(sc["pair_base"][W_LO[k + 1]])
                    npc_ = p_hi - p_lo
                    sb = pool.tile([128, npc_, WSEG], bf, tag=f"s{x}",
                                   name=f"sbuf{x}")
                    col0 = (0 if x == 0 else npA) + p_lo
                    for p0 in range(0, npc_, 32):
                        p1 = min(p0 + 32, npc_)
                        nc.sync.dma_start(
                            out=sb[:, p0:p1, :],
                            in_=t_S[:, (col0 + p0) * WSEG : (col0 + p1) * WSEG])
                    sbufs.append(sb)
                chunk_bufs[k] = (bufs, sbufs)

            def emit_matmuls(k):
                bufs, sbufs = chunk_bufs[k]
                prev = chunk_bufs.get(k - 1)
                w_lo, w_hi = int(W_LO[k]), int(W_LO[k + 1])
                for w in range(w_lo, w_hi):
                    psum = psw.tile([D, WSEG], f32, space="PSUM", tag="pw")
                    spans = [(int(sched[x]["ts"][w]), int(sched[x]["te"][w]))
                             for x in range(2)]
                    total_t = sum(e - s for s, e in spans)
                    ti = 0
                    for x in range(2):
                        sc = sched[x]
                        p_lo = int(sc["pair_base"][W_LO[k]])
                        for t in range(*spans[x]):
                            if t < g[x, k]:
                                gb = prev[0][x]
                                gt = t - int(g[x, k - 1])
                            else:
                                gb = bufs[x]
                                gt = t - int(g[x, k])
                            pair = (int(sc["pair_base"][w])
                                    + (t - int(sc["ts"][w])) - p_lo)
                            nc.tensor.matmul(
                                psum[:],
                                lhsT=gb[:, gt, :],
                                rhs=sbufs[x][:, pair, :],
                                start=(ti == 0),
                                stop=(ti == total_t - 1),
                            )
                            ti += 1
                    o = w * WSEG
                    e = o + WSEG
                    braw = brawp.tile([D, WSEG], bf, tag="br")
                    nc.scalar.activation(braw[:], psum[:],
                                         mybir.ActivationFunctionType.Copy)
                    x1 = px1p.tile([D, WSEG], bf, tag="x1")
                    x2 = px2p.tile([D, WSEG], bf, tag="x2")
                    nc.vector.tensor_tensor(
                        out=x1[:], in0=embT[:, o:e], in1=braw[:],
                        op=mybir.AluOpType.add)
                    nc.vector.tensor_tensor(
                        out=x2[:], in0=embT[:, o:e], in1=braw[:],
                        op=mybir.AluOpType.mult)
                    win_x[w] = (x1, x2)
                chunk_bufs.pop(k - 1, None)

            def emit_downstream(k):
                w_lo, w_hi = int(W_LO[k]), int(W_LO[k + 1])
                for w in range(w_lo, w_hi):
                    x1, x2 = win_x.pop(w)
                    r0 = w * 128
                    nrow = min(128, NPC - r0)
                    p1 = psd.tile([128, D], f32, space="PSUM", tag="pd")
                    nc.tensor.matmul(p1[:nrow, :], lhsT=x1[:, :nrow],
                                     rhs=wsum[:], start=True, stop=True)
                    p2 = psd.tile([128, D], f32, space="PSUM", tag="pd")
                    nc.tensor.matmul(p2[:nrow, :], lhsT=x2[:, :nrow],
                                     rhs=wprod[:], start=True, stop=True)
                    t1 = small.tile([128, D], f32, tag="t1")
                    t2 = small.tile([128, D], f32, tag="t2")
                    if with_bias:
                        b1 = small.tile([128, D], f32, tag="b1")
                        b2 = small.tile([128, D], f32, tag="b2")
                        nc.vector.tensor_tensor(out=b1[:nrow, :],
                                                in0=p1[:nrow, :],
                                                in1=bias[:nrow, :D],
                                                op=mybir.AluOpType.add)
                        nc.vector.tensor_tensor(out=b2[:nrow, :],
                                                in0=p2[:nrow, :],
                                                in1=bias[:nrow, D:],
                                                op=mybir.AluOpType.add)
                        nc.scalar.activation(t1[:nrow, :], b1[:nrow, :],
                                             **_act_kw())
                        nc.scalar.activation(t2[:nrow, :], b2[:nrow, :],
                                             **_act_kw())
                    else:
                        nc.scalar.activation(t1[:nrow, :], p1[:nrow, :],
                                             **_act_kw())
                        nc.scalar.activation(t2[:nrow, :], p2[:nrow, :],
                                             **_act_kw())
                    ob = small.tile([128, D], f32, tag="ob")
                    nc.vector.tensor_tensor(out=ob[:nrow, :], in0=t1[:nrow, :],
                                            in1=t2[:nrow, :],
                                            op=mybir.AluOpType.add)
                    nc.sync.dma_start(out=t_out[r0 : r0 + nrow, :],
                                      in_=ob[:nrow, :])

            NB = math.ceil(NPC / 128)  # 49 blocks, last partial (106 rows)

            def emit_embt(k):
                """PE-transpose embT blocks for chunk k's windows."""
                for b in range(int(W_LO[k]), int(W_LO[k + 1])):
                    r0 = b * 128
                    nrow = min(128, NPC - r0)
                    eb = small.tile([128, D], bf, tag="eb")
                    if nrow < 128:
                        nc.vector.memset(eb[:], 0)
                    nc.sync.dma_start(out=eb[:nrow, :],
                                      in_=t_emb[r0 : r0 + nrow, :])
                    pt = pst.tile([D, 128], bf, space="PSUM", tag="pt")
                    nc.tensor.transpose(pt[:], eb[:], ident[:])
                    nc.scalar.activation(embT[:, r0 : r0 + 128], pt[:],
                                         mybir.ActivationFunctionType.Copy)

            # ---- chunk-0 gathers launch first; bulk idx right behind
            emit_gather_and_s(0)
            emit_idx_rest()

            # ---- remaining statics behind chunk-0's gathers
            wsum = cst.tile([D, D], bf)
            nc.sync.dma_start(out=wsum[:], in_=t_wsum[:])
            wprod = cst.tile([D, D], bf)
            nc.sync.dma_start(out=wprod[:], in_=t_wprod[:])
            if with_bias:
                bias = cst.tile([128, 2 * D], f32)
                nc.sync.dma_start(out=bias[:], in_=t_bias[:])
            ident = cst.tile([128, 128], bf)
            make_identity(nc, ident[:])
            embT = cst.tile([D, NCOL], bf)
            emit_embt(0)

            # ---- software pipeline over chunks
            for k in range(1, NCHUNK + 2):
                if k < NCHUNK:
                    emit_gather_and_s(k)
                    emit_embt(k)
                if k >= 2:
                    emit_downstream(k - 2)
                if 1 <= k <= NCHUNK:
                    emit_matmuls(k - 1)

    return nc


def kernel(embeddings, edge_row, edge_col, edge_val, W_sum, b_sum, W_prod,
           b_prod, _debug=False, _trace=False):
    embeddings = np.asarray(embeddings)
    edge_row = np.asarray(edge_row).astype(np.int64)
    edge_col = np.asarray(edge_col).astype(np.int64)
    edge_val = np.asarray(edge_val)
    W_sum = np.asarray(W_sum)
    W_prod = np.asarray(W_prod)
    b_sum = np.asarray(b_sum)
    b_prod = np.asarray(b_prod)

    per_core, sched, core_dests = _preprocess(edge_row, edge_col, edge_val)
    with_bias = bool(np.any(b_sum) or np.any(b_prod))

    table_bf = embeddings.astype(BF16)
    wsumT = np.ascontiguousarray(W_sum.T).astype(BF16)
    wprodT = np.ascontiguousarray(W_prod.T).astype(BF16)
    if with_bias:
        biases = np.concatenate(
            [np.tile(b_sum[None, :], (128, 1)),
             np.tile(b_prod[None, :], (128, 1))], axis=1).astype(np.float32)

    nc = bacc.Bacc(num_swdge_queues=4, dynamic_dma_scratch_size=SCRATCH)
    _build(nc, sched, with_bias)
    nc.compile()

    in_maps = []
    for c in range(NCORES):
        m = dict(
            table=np.asarray(table_bf),
            emb_own=np.ascontiguousarray(table_bf[core_dests[c]]),
            idxA=per_core[c]["idxA"],
            idxB=per_core[c]["idxB"],
            S=per_core[c]["S"],
            wsumT=np.asarray(wsumT),
            wprodT=np.asarray(wprodT),
        )
        if with_bias:
            m["biases"] = biases
        in_maps.append(m)

    res = run_bass_kernel_spmd(nc, in_maps, list(range(NCORES)),
                               trace=_trace)
    out = np.empty((N_NODES, D), dtype=np.float32)
    for c in range(NCORES):
        out[core_dests[c]] = res.results[c]["out"]
    if _debug:
        return out, res
    return out

